# revision 17
# baseline (speedup 1.0000x reference)
"""Trainium2 Bass kernel for the 2-layer Mamba net (nn_Net_18425409700140).

Data-parallel over batch: 32 batches -> 8 cores x 4 batches.
Per core, per layer: model-dim ops in [feature, token] layout, selective-scan
state expanded to 4 partition-tiles of [(e_local 8, n 16)=128, token], scan via
DVE tensor_tensor_scan, n-contraction via PE selection matmuls.
"""
import numpy as np

import concourse.bacc as bacc
import concourse.mybir as mybir
from concourse.tile import TileContext
from concourse.bass_utils import run_bass_kernel_spmd

FP32 = mybir.dt.float32
AL = mybir.AluOpType
AF = mybir.ActivationFunctionType

B_LOC = 4          # batches per core
L = 4096           # sequence length
T = 512            # chunk (tokens per inner step)
DM = 16            # d_model
ED = 32            # d_inner
N = 16             # d_state
NCHUNK = L // T
EPS = 1e-5


def build_nc(b_loc=B_LOC, seq=L):
    nchunk = seq // T
    nc = bacc.Bacc("TRN2", target_bir_lowering=False, debug=False,
                   enable_asserts=True, num_devices=1)

    # ---------------- DRAM tensors ----------------
    x_in = nc.dram_tensor("x", [b_loc, seq, 64], FP32, kind="ExternalInput")
    out_d = nc.dram_tensor("out", [b_loc, seq], FP32, kind="ExternalOutput")

    def din(name, shape):
        return nc.dram_tensor(name, shape, FP32, kind="ExternalInput")

    win_T = din("win_T", [64, DM])          # lin_in_w.T
    win_b = din("win_b", [DM, 1])           # lin_in bias (per-partition col)
    wlo_T = din("wlo_T", [DM, 1])           # lin_out_w.T
    lob = din("lob", [1, 1])                # lin_out bias
    ident = din("ident", [128, 128])
    ones16 = din("ones16", [DM, 1])         # column of ones (lhsT for ms)
    ones_1x16 = din("ones_1x16", [1, DM])
    ones_1x32 = din("ones_1x32", [1, ED])
    selE = din("selE", [4, ED, 128])        # lhsT: delta/u broadcast per eo
    selB = din("selB", [N, 128])            # lhsT: B/C broadcast (n = p%16)
    selS = din("selS", [4, 128, ED])        # lhsT: y reduce per eo
    # per-layer packed weights
    wcz = din("wcz", [2, 4, DM, 2 * ED])    # conv-fused in_proj, 4 shifts
    sbias = din("sbias", [2, 2 * ED, 1])    # [conv_b; zeros] silu bias
    wx_T = din("wx_T", [2, ED, 80])         # x_proj.T row-padded: r@0,B@32,C@64
    dtw = din("dtw", [2, ED, 1])
    dtb = din("dtb", [2, ED, 1])
    acol = din("acol", [2, 4, 128, 1])      # A[e(p), n(p)] per eo
    dvec = din("dvec", [2, ED, 1])          # D
    wout_T = din("wout_T", [2, ED, DM])     # out_proj.T
    cst = din("cst", [128, 2])              # col0: eps, col1: 1/DM

    with TileContext(nc) as tc:
        with (
            tc.tile_pool(name="pw", bufs=1) as pw,
            tc.tile_pool(name="pers", bufs=1) as pers,
            tc.tile_pool(name="work", bufs=3) as wk,
            tc.tile_pool(name="wkbig", bufs=3) as wkb,
            tc.tile_pool(name="ppbig", bufs=2, space="PSUM") as ppb,
            tc.tile_pool(name="ppbc", bufs=2, space="PSUM") as ppc,
            tc.tile_pool(name="ppmid", bufs=1, space="PSUM") as ppm,
            tc.tile_pool(name="ppsml", bufs=2, space="PSUM") as pps,
            tc.tile_pool(name="ppy", bufs=1, space="PSUM") as ppy,
        ):
            # ---------- load weights/constants ----------
            def wtile(ap, tag):
                t = pw.tile(list(ap.shape), FP32, tag=tag, name=tag)
                nc.sync.dma_start(t[:], ap[:])
                return t

            t_winT = wtile(win_T.ap(), "winT")
            t_winb = wtile(win_b.ap(), "winb")
            t_wloT = wtile(wlo_T.ap(), "wloT")
            t_lob = wtile(lob.ap(), "lob")
            t_ident = wtile(ident.ap(), "ident")
            t_ones16 = wtile(ones16.ap(), "ones16")
            t_o1x16 = wtile(ones_1x16.ap(), "o1x16")
            t_o1x32 = wtile(ones_1x32.ap(), "o1x32")
            t_selB32 = pw.tile([48, 128], FP32, tag="selB32", name="selB32")
            nc.sync.dma_start(t_selB32[32:48, :], selB[:])
            t_selB64 = pw.tile([80, 128], FP32, tag="selB64", name="selB64")
            nc.sync.dma_start(t_selB64[64:80, :], selB[:])
            t_selE = [wtile(selE[eo], f"selE{eo}") for eo in range(4)]
            t_selS = [wtile(selS[eo], f"selS{eo}") for eo in range(4)]
            t_wcz = [[wtile(wcz[l, k], f"wcz{l}{k}") for k in range(4)]
                     for l in range(2)]
            t_sbias = [wtile(sbias[l], f"sbias{l}") for l in range(2)]
            t_wxT = [wtile(wx_T[l], f"wxT{l}") for l in range(2)]
            t_dtw = [wtile(dtw[l], f"dtw{l}") for l in range(2)]
            t_dtb = [wtile(dtb[l], f"dtb{l}") for l in range(2)]
            t_acol = [[wtile(acol[l, eo], f"acol{l}{eo}") for eo in range(4)]
                      for l in range(2)]
            t_dvec = [wtile(dvec[l], f"dvec{l}") for l in range(2)]
            t_woutT = [wtile(wout_T[l], f"woutT{l}") for l in range(2)]
            t_cst = wtile(cst.ap(), "cst")

            # ---------- persistent buffers ----------
            t_res = [pers.tile([DM, seq], FP32, tag=f"res{b}", name=f"res{b}")
                     for b in range(b_loc)]
            t_carry = [[pers.tile([128, 1], FP32, tag=f"carry{b}_{eo}", name=f"carry{b}_{eo}")
                        for eo in range(4)] for b in range(b_loc)]

            # ---------- prologue: x load + transpose + lin_in ----------
            for b in range(b_loc):
                for c in range(nchunk):
                    # contiguous 128KB load: tile[p, j, f] = x[b, t0 + 4p + j, f]
                    xt = wk.tile([128, 4, 64], FP32, tag="xload")
                    nc.sync.dma_start(
                        xt[:], x_in[b, c * T:(c + 1) * T, :].rearrange(
                            "(p j) f -> p j f", p=128))
                    pt = ppm.tile([64, T], FP32, tag="pmid")
                    for j in range(4):
                        nc.tensor.transpose(
                            pt[:].rearrange("e (tp j) -> e tp j", j=4)[:, :, j],
                            xt[:, j, :], t_ident[:])
                    xTc = wk.tile([64, T], FP32, tag="xTc")
                    nc.vector.tensor_copy(xTc[:], pt[:])
                    ph = pps.tile([DM, T], FP32, tag="ps_small")
                    nc.tensor.matmul(ph[:], lhsT=t_winT[:], rhs=xTc[:],
                                     start=True, stop=True)
                    nc.scalar.activation(t_res[b][:, c * T:(c + 1) * T],
                                         ph[:], AF.Identity, bias=t_winb[:])

            # ---------- layers ----------
            for l in range(2):
                r_in = t_res
                r_out = t_res
                for b in range(b_loc):
                    t_hnb = wkb.tile([DM, seq + 3], FP32, tag="hnb", bufs=1,
                                     name=f"hnb{l}{b}")

                    # reset hn pad + carries
                    nc.vector.memset(t_hnb[:, 0:3], 0.0)
                    for eo in range(4):
                        nc.vector.memset(t_carry[b][eo][:], 0.0)

                    for c in range(nchunk):
                        s0, s1 = c * T, (c + 1) * T
                        # rms: ms -> rs = exp(-0.5 ln(ms/16 + eps))
                        sq = wk.tile([DM, T], FP32, tag="sq")
                        nc.scalar.activation(
                            sq[:], r_in[b][:, s0:s1], AF.Square)
                        pms = pps.tile([1, T], FP32, tag="ps_small")
                        nc.tensor.matmul(pms[:], lhsT=t_ones16[:], rhs=sq[:],
                                         start=True, stop=True)
                        lms = wk.tile([1, T], FP32, tag="lms")
                        nc.scalar.activation(lms[:], pms[:], AF.Ln,
                                             scale=1.0 / DM,
                                             bias=t_cst[0:1, 0:1])
                        rsc = wk.tile([1, T], FP32, tag="rsc")
                        nc.scalar.activation(rsc[:], lms[:], AF.Exp,
                                             scale=-0.5)
                        prs = pps.tile([DM, T], FP32, tag="ps_small")
                        nc.tensor.matmul(prs[:], lhsT=t_o1x16[:],
                                         rhs=rsc[:],
                                         start=True, stop=True)
                        nc.vector.tensor_tensor(
                            t_hnb[:, 3 + s0:3 + s1],
                            r_in[b][:, s0:s1], prs[:], AL.mult)
                        # conv + z: 4 shifted matmuls
                        pcz = ppm.tile([2 * ED, T], FP32, tag="pmid")
                        for k in range(4):
                            nc.tensor.matmul(
                                pcz[:], lhsT=t_wcz[l][k][:],
                                rhs=t_hnb[:, s0 + k:s0 + k + T],
                                start=(k == 0), stop=(k == 3))
                        sc = wk.tile([2 * ED, T], FP32, tag="sc")
                        nc.scalar.activation(sc[:], pcz[:], AF.Silu,
                                             bias=t_sbias[l][:])
                        # x_proj
                        pdbc = pps.tile([80, T], FP32, tag="ps_small")
                        nc.tensor.matmul(pdbc[:], lhsT=t_wxT[l][:],
                                         rhs=sc[0:ED, :], start=True, stop=True)
                        dbc = wk.tile([80, T], FP32, tag="dbc")
                        nc.scalar.activation(dbc[:], pdbc[:], AF.Copy)
                        # delta = softplus(dtw * r + dtb)
                        pr = pps.tile([ED, T], FP32, tag="ps_small")
                        nc.tensor.matmul(pr[:], lhsT=t_o1x32[:], rhs=dbc[0:1, :],
                                         start=True, stop=True)
                        ez = wk.tile([ED, T], FP32, tag="ez")
                        nc.scalar.activation(ez[:], pr[:], AF.Exp,
                                             scale=t_dtw[l][:], bias=t_dtb[l][:])
                        delta = wk.tile([ED, T], FP32, tag="delta")
                        nc.scalar.activation(delta[:], ez[:], AF.Ln, bias=1.0)
                        # u = delta * xc
                        u = wk.tile([ED, T], FP32, tag="u")
                        nc.vector.tensor_tensor(u[:], delta[:], sc[0:ED, :],
                                                AL.mult)
                        # B_bc / C_bc (shared across eo)
                        pB = ppc.tile([128, T], FP32, tag="pbc")
                        nc.tensor.matmul(pB[:], lhsT=t_selB32[32:48, :], rhs=dbc[32:48, :],
                                         start=True, stop=True)
                        Bbc = wkb.tile([128, T], FP32, tag="Bbc")
                        nc.scalar.activation(Bbc[:], pB[:], AF.Copy)
                        pC = ppc.tile([128, T], FP32, tag="pbc")
                        nc.tensor.matmul(pC[:], lhsT=t_selB64[64:80, :], rhs=dbc[64:80, :],
                                         start=True, stop=True)
                        Cbc = wkb.tile([128, T], FP32, tag="Cbc")
                        nc.scalar.activation(Cbc[:], pC[:], AF.Copy)

                        py = ppy.tile([ED, T], FP32, tag="psy")
                        for eo in range(4):
                            pdb = ppb.tile([128, T], FP32, tag="pdb")
                            nc.tensor.matmul(pdb[:], lhsT=t_selE[eo][:],
                                             rhs=delta[:], start=True, stop=True)
                            dA = wkb.tile([128, T], FP32, tag="dA")
                            nc.scalar.activation(dA[:], pdb[:], AF.Exp,
                                                 scale=t_acol[l][eo][:])
                            pub = ppb.tile([128, T], FP32, tag="pdb")
                            nc.tensor.matmul(pub[:], lhsT=t_selE[eo][:],
                                             rhs=u[:], start=True, stop=True)
                            BX = wkb.tile([128, T], FP32, tag="BX")
                            nc.vector.tensor_tensor(BX[:], pub[:], Bbc[:],
                                                    AL.mult)
                            h = wkb.tile([128, T], FP32, tag="h")
                            nc.vector.tensor_tensor_scan(
                                out=h[:], data0=dA[:], data1=BX[:],
                                initial=t_carry[b][eo][:],
                                op0=AL.mult, op1=AL.add)
                            nc.vector.tensor_copy(t_carry[b][eo][:],
                                                  h[:, T - 1:T])
                            Hc = wkb.tile([128, T], FP32, tag="Hc")
                            nc.vector.tensor_tensor(Hc[:], h[:], Cbc[:], AL.mult)
                            nc.tensor.matmul(py[:], lhsT=t_selS[eo][:],
                                             rhs=Hc[:], start=(eo == 0),
                                             stop=(eo == 3))
                        # y2 = xc*D + y ; gated = y2 * silu_z; out_proj + resid
                        y2 = wk.tile([2 * ED, T], FP32, tag="y2")
                        nc.vector.scalar_tensor_tensor(
                            out=y2[ED:, :], in0=sc[0:ED, :], scalar=t_dvec[l][:],
                            in1=py[:], op0=AL.mult, op1=AL.add)
                        gated = wk.tile([ED, T], FP32, tag="gated")
                        nc.vector.tensor_tensor(gated[:], y2[ED:, :], sc[ED:, :],
                                                AL.mult)
                        po = pps.tile([DM, T], FP32, tag="ps_small")
                        nc.tensor.matmul(po[:], lhsT=t_woutT[l][:], rhs=gated[:],
                                         start=True, stop=True)
                        nc.vector.tensor_tensor(r_out[b][:, s0:s1], po[:],
                                                r_in[b][:, s0:s1], AL.add)

            # ---------- epilogue: lin_out ----------
            for b in range(b_loc):
                for c in range(nchunk):
                    pf = pps.tile([1, T], FP32, tag="ps_small")
                    nc.tensor.matmul(pf[:], lhsT=t_wloT[:],
                                     rhs=t_res[b][:, c * T:(c + 1) * T],
                                     start=True, stop=True)
                    oc = wk.tile([1, T], FP32, tag="oc")
                    nc.scalar.activation(oc[:], pf[:], AF.Identity,
                                         bias=t_lob[:])
                    nc.sync.dma_start(out_d[b:b + 1, c * T:(c + 1) * T], oc[:])

    nc.finalize()
    return nc


def pack_weights(inputs):
    """Host-side packing of the tiny weight set into device layouts."""
    f = np.float32
    w = {}
    w["win_T"] = np.ascontiguousarray(inputs["lin_in_w"].T).astype(f)
    w["win_b"] = inputs["lin_in_b"].reshape(DM, 1).astype(f)
    w["wlo_T"] = np.ascontiguousarray(inputs["lin_out_w"].T).astype(f)
    w["lob"] = inputs["lin_out_b"].reshape(1, 1).astype(f)
    w["ident"] = np.eye(128, dtype=f)
    w["ones16"] = np.ones((DM, 1), f)
    w["ones_1x16"] = np.ones((1, DM), f)
    w["ones_1x32"] = np.ones((1, ED), f)
    selE = np.zeros((4, ED, 128), f)
    selS = np.zeros((4, 128, ED), f)
    for eo in range(4):
        for p in range(128):
            e = eo * 8 + p // 16
            selE[eo, e, p] = 1.0
            selS[eo, p, e] = 1.0
    w["selE"] = selE
    w["selS"] = selS
    selB = np.zeros((N, 128), f)
    for p in range(128):
        selB[p % 16, p] = 1.0
    w["selB"] = selB

    wcz = np.zeros((2, 4, DM, 2 * ED), f)
    sbias = np.zeros((2, 2 * ED, 1), f)
    wxT = np.zeros((2, ED, 80), f)
    dtw = np.zeros((2, ED, 1), f)
    dtb = np.zeros((2, ED, 1), f)
    acol = np.zeros((2, 4, 128, 1), f)
    dvec = np.zeros((2, ED, 1), f)
    woutT = np.zeros((2, ED, DM), f)
    for l in range(2):
        p = f"l{l}_"
        in_proj = inputs[p + "in_proj_w"].astype(f)      # [64, 16]
        norm_w = inputs[p + "norm_w"].astype(f)          # [16]
        ipw = in_proj * norm_w[None, :]                  # fold norm weight
        conv_w = inputs[p + "conv_w"].astype(f)          # [32, 4]
        for k in range(4):
            # xin part: rows of in_proj 0..31, scaled by conv_w[:, k]
            wcz[l, k, :, 0:ED] = (ipw[0:ED, :] * conv_w[:, k:k + 1]).T
            if k == 3:
                wcz[l, k, :, ED:] = ipw[ED:, :].T        # z passthrough
        sbias[l, 0:ED, 0] = inputs[p + "conv_b"].astype(f)
        xp = inputs[p + "x_proj_w"].astype(f)            # [33, 32]
        wxT[l][:, 0:1] = xp[0:1, :].T                    # r
        wxT[l][:, 32:48] = xp[1:17, :].T                 # B
        wxT[l][:, 64:80] = xp[17:33, :].T                # C
        dtw[l] = inputs[p + "dt_proj_w"].astype(f).reshape(ED, 1)
        dtb[l] = inputs[p + "dt_proj_b"].astype(f).reshape(ED, 1)
        A = -np.exp(inputs[p + "A_log"].astype(np.float64)).astype(f)  # [32,16]
        for eo in range(4):
            for q in range(128):
                acol[l, eo, q, 0] = A[eo * 8 + q // 16, q % 16]
        dvec[l] = inputs[p + "D"].astype(f).reshape(ED, 1)
        woutT[l] = inputs[p + "out_proj_w"].astype(f).T  # [32, 16]
    w["wcz"], w["sbias"], w["wx_T"] = wcz, sbias, wxT
    c2 = np.zeros((128, 2), f); c2[:, 0] = EPS; c2[:, 1] = 1.0 / DM
    w["cst"] = c2
    w["dtw"], w["dtb"], w["acol"], w["dvec"], w["wout_T"] = (
        dtw, dtb, acol, dvec, woutT)
    return w


_NC_CACHE = {}


def get_nc():
    if "nc" not in _NC_CACHE:
        _NC_CACHE["nc"] = build_nc()
    return _NC_CACHE["nc"]


def kernel(**inputs):
    nc = get_nc()
    w = pack_weights(inputs)
    x = np.asarray(inputs["x"], np.float32)          # [32, 4096, 64]
    n_cores = 8
    in_maps = []
    for c in range(n_cores):
        m = dict(w)
        m["x"] = np.ascontiguousarray(x[c * B_LOC:(c + 1) * B_LOC])
        in_maps.append(m)
    res = run_bass_kernel_spmd(nc, in_maps, list(range(n_cores)))
    outs = [res.results[c]["out"] for c in range(n_cores)]   # each [4, 4096]
    full = np.concatenate(outs, axis=0)                       # [32, 4096]
    return full.reshape(-1).astype(np.float32)


# revision 19
# speedup vs baseline: 4.2564x; 4.2564x over previous
"""Trainium2 Bass kernel for the 2-layer Mamba net (nn_Net_18425409700140).

Data-parallel over batch: 32 batches -> 8 cores x 4 batches.
Per core, per layer: model-dim ops in [feature, token] layout, selective-scan
state expanded to 4 partition-tiles of [(e_local 8, n 16)=128, token], scan via
DVE tensor_tensor_scan, n-contraction via PE selection matmuls.
"""
import numpy as np

import concourse.bacc as bacc
import concourse.mybir as mybir
from concourse.tile import TileContext
from concourse.bass_utils import run_bass_kernel_spmd

FP32 = mybir.dt.float32
AL = mybir.AluOpType
AF = mybir.ActivationFunctionType

B_LOC = 4          # batches per core
L = 4096           # sequence length
T = 512            # chunk (tokens per inner step)
DM = 16            # d_model
ED = 32            # d_inner
N = 16             # d_state
NCHUNK = L // T
EPS = 1e-5


def build_nc(b_loc=B_LOC, seq=L):
    nchunk = seq // T
    nc = bacc.Bacc("TRN2", target_bir_lowering=False, debug=False,
                   enable_asserts=True, num_devices=1)

    # ---------------- DRAM tensors ----------------
    x_in = nc.dram_tensor("x", [b_loc, seq, 64], FP32, kind="ExternalInput")
    out_d = nc.dram_tensor("out", [b_loc, seq], FP32, kind="ExternalOutput")

    def din(name, shape):
        return nc.dram_tensor(name, shape, FP32, kind="ExternalInput")

    win_T = din("win_T", [64, DM])          # lin_in_w.T
    win_b = din("win_b", [DM, 1])           # lin_in bias (per-partition col)
    wlo_T = din("wlo_T", [DM, 1])           # lin_out_w.T
    lob = din("lob", [1, 1])                # lin_out bias
    ident = din("ident", [128, 128])
    ones16 = din("ones16", [DM, 1])         # column of ones (lhsT for ms)
    ones_1x16 = din("ones_1x16", [1, DM])
    ones_1x32 = din("ones_1x32", [1, ED])
    selE = din("selE", [4, ED, 128])        # lhsT: delta/u broadcast per eo
    selB = din("selB", [N, 128])            # lhsT: B/C broadcast (n = p%16)
    selS = din("selS", [4, 128, ED])        # lhsT: y reduce per eo
    # per-layer packed weights
    wcz = din("wcz", [2, 4, DM, 2 * ED])    # conv-fused in_proj, 4 shifts
    sbias = din("sbias", [2, 2 * ED, 1])    # [conv_b; zeros] silu bias
    wx_T = din("wx_T", [2, ED, 80])         # x_proj.T row-padded: r@0,B@32,C@64
    dtw = din("dtw", [2, ED, 1])
    dtb = din("dtb", [2, ED, 1])
    acol = din("acol", [2, 4, 128, 1])      # A[e(p), n(p)] per eo
    dvec = din("dvec", [2, ED, 1])          # D
    wout_T = din("wout_T", [2, ED, DM])     # out_proj.T
    cst = din("cst", [128, 2])              # col0: eps, col1: 1/DM

    with TileContext(nc) as tc:
        with (
            tc.tile_pool(name="pw", bufs=1) as pw,
            tc.tile_pool(name="pers", bufs=1) as pers,
            tc.tile_pool(name="work", bufs=3) as wk,
            tc.tile_pool(name="wkbig", bufs=3) as wkb,
            tc.tile_pool(name="ppbig", bufs=2, space="PSUM") as ppb,
            tc.tile_pool(name="ppbc", bufs=2, space="PSUM") as ppc,
            tc.tile_pool(name="ppmid", bufs=1, space="PSUM") as ppm,
            tc.tile_pool(name="ppsml", bufs=2, space="PSUM") as pps,
            tc.tile_pool(name="ppy", bufs=1, space="PSUM") as ppy,
        ):
            # ---------- load weights/constants ----------
            def wtile(ap, tag):
                t = pw.tile(list(ap.shape), FP32, tag=tag, name=tag)
                nc.sync.dma_start(t[:], ap[:])
                return t

            t_winT = wtile(win_T.ap(), "winT")
            t_winb = wtile(win_b.ap(), "winb")
            t_wloT = wtile(wlo_T.ap(), "wloT")
            t_lob = wtile(lob.ap(), "lob")
            t_ident = wtile(ident.ap(), "ident")
            t_ones16 = wtile(ones16.ap(), "ones16")
            t_o1x16 = wtile(ones_1x16.ap(), "o1x16")
            t_o1x32 = wtile(ones_1x32.ap(), "o1x32")
            t_selB32 = pw.tile([48, 128], FP32, tag="selB32", name="selB32")
            nc.sync.dma_start(t_selB32[32:48, :], selB[:])
            t_selB64 = pw.tile([80, 128], FP32, tag="selB64", name="selB64")
            nc.sync.dma_start(t_selB64[64:80, :], selB[:])
            t_selE = [wtile(selE[eo], f"selE{eo}") for eo in range(4)]
            t_selS = [wtile(selS[eo], f"selS{eo}") for eo in range(4)]
            t_wcz = [[wtile(wcz[l, k], f"wcz{l}{k}") for k in range(4)]
                     for l in range(2)]
            t_sbias = [wtile(sbias[l], f"sbias{l}") for l in range(2)]
            t_wxT = [wtile(wx_T[l], f"wxT{l}") for l in range(2)]
            t_dtw = [wtile(dtw[l], f"dtw{l}") for l in range(2)]
            t_dtb = [wtile(dtb[l], f"dtb{l}") for l in range(2)]
            t_acol = [[wtile(acol[l, eo], f"acol{l}{eo}") for eo in range(4)]
                      for l in range(2)]
            t_dvec = [wtile(dvec[l], f"dvec{l}") for l in range(2)]
            t_woutT = [wtile(wout_T[l], f"woutT{l}") for l in range(2)]
            t_cst = wtile(cst.ap(), "cst")

            # ---------- persistent buffers ----------
            t_res = [pers.tile([DM, seq], FP32, tag=f"res{b}", name=f"res{b}")
                     for b in range(b_loc)]
            t_carry = [[pers.tile([128, 1], FP32, tag=f"carry{b}_{eo}", name=f"carry{b}_{eo}")
                        for eo in range(4)] for b in range(b_loc)]

            # ---------- prologue: x load + transpose + lin_in ----------
            for b in range(b_loc):
                for c in range(nchunk):
                    # contiguous 128KB load: tile[p, j, f] = x[b, t0 + 4p + j, f]
                    xt = wk.tile([128, 4, 64], FP32, tag="xload")
                    nc.sync.dma_start(
                        xt[:], x_in[b, c * T:(c + 1) * T, :].rearrange(
                            "(p j) f -> p j f", p=128))
                    pt = ppm.tile([64, T], FP32, tag="pmid")
                    for j in range(4):
                        nc.tensor.transpose(
                            pt[:].rearrange("e (tp j) -> e tp j", j=4)[:, :, j],
                            xt[:, j, :], t_ident[:])
                    xTc = wk.tile([64, T], FP32, tag="xTc")
                    nc.vector.tensor_copy(xTc[:], pt[:])
                    ph = pps.tile([DM, T], FP32, tag="ps_small")
                    nc.tensor.matmul(ph[:], lhsT=t_winT[:], rhs=xTc[:],
                                     start=True, stop=True)
                    nc.scalar.activation(t_res[b][:, c * T:(c + 1) * T],
                                         ph[:], AF.Identity, bias=t_winb[:])

            # ---------- layers ----------
            for l in range(2):
                r_in = t_res
                r_out = t_res
                for b in range(b_loc):
                    t_hnb = wkb.tile([DM, seq + 3], FP32, tag="hnb", bufs=1,
                                     name=f"hnb{l}{b}")

                    # reset hn pad + carries
                    nc.vector.memset(t_hnb[:, 0:3], 0.0)
                    for eo in range(4):
                        nc.vector.memset(t_carry[b][eo][:], 0.0)

                    for c in range(nchunk):
                        s0, s1 = c * T, (c + 1) * T
                        # rms: ms -> rs = exp(-0.5 ln(ms/16 + eps))
                        sq = wk.tile([DM, T], FP32, tag="sq")
                        nc.scalar.activation(
                            sq[:], r_in[b][:, s0:s1], AF.Square)
                        pms = pps.tile([1, T], FP32, tag="ps_small")
                        nc.tensor.matmul(pms[:], lhsT=t_ones16[:], rhs=sq[:],
                                         start=True, stop=True)
                        lms = wk.tile([1, T], FP32, tag="lms")
                        nc.scalar.activation(lms[:], pms[:], AF.Ln,
                                             scale=1.0 / DM,
                                             bias=t_cst[0:1, 0:1])
                        rsc = wk.tile([1, T], FP32, tag="rsc")
                        nc.scalar.activation(rsc[:], lms[:], AF.Exp,
                                             scale=-0.5)
                        prs = pps.tile([DM, T], FP32, tag="ps_small")
                        nc.tensor.matmul(prs[:], lhsT=t_o1x16[:],
                                         rhs=rsc[:],
                                         start=True, stop=True)
                        nc.vector.tensor_tensor(
                            t_hnb[:, 3 + s0:3 + s1],
                            r_in[b][:, s0:s1], prs[:], AL.mult)
                        # conv + z: 4 shifted matmuls
                        pcz = ppm.tile([2 * ED, T], FP32, tag="pmid")
                        for k in range(4):
                            nc.tensor.matmul(
                                pcz[:], lhsT=t_wcz[l][k][:],
                                rhs=t_hnb[:, s0 + k:s0 + k + T],
                                start=(k == 0), stop=(k == 3))
                        sc = wk.tile([2 * ED, T], FP32, tag="sc")
                        nc.scalar.activation(sc[:], pcz[:], AF.Silu,
                                             bias=t_sbias[l][:])
                        # x_proj
                        pdbc = pps.tile([80, T], FP32, tag="ps_small")
                        nc.tensor.matmul(pdbc[:], lhsT=t_wxT[l][:],
                                         rhs=sc[0:ED, :], start=True, stop=True)
                        dbc = wk.tile([80, T], FP32, tag="dbc")
                        nc.scalar.activation(dbc[:], pdbc[:], AF.Copy)
                        # delta = softplus(dtw * r + dtb)
                        pr = pps.tile([ED, T], FP32, tag="ps_small")
                        nc.tensor.matmul(pr[:], lhsT=t_o1x32[:], rhs=dbc[0:1, :],
                                         start=True, stop=True)
                        ez = wk.tile([ED, T], FP32, tag="ez")
                        nc.scalar.activation(ez[:], pr[:], AF.Exp,
                                             scale=t_dtw[l][:], bias=t_dtb[l][:])
                        delta = wk.tile([ED, T], FP32, tag="delta")
                        nc.scalar.activation(delta[:], ez[:], AF.Ln, bias=1.0)
                        # u = delta * xc
                        u = wk.tile([ED, T], FP32, tag="u")
                        nc.vector.tensor_tensor(u[:], delta[:], sc[0:ED, :],
                                                AL.mult)
                        # B_bc / C_bc (shared across eo)
                        pB = ppc.tile([128, T], FP32, tag="pbc")
                        nc.tensor.matmul(pB[:], lhsT=t_selB32[32:48, :], rhs=dbc[32:48, :],
                                         start=True, stop=True)
                        Bbc = wkb.tile([128, T], FP32, tag="Bbc")
                        nc.scalar.activation(Bbc[:], pB[:], AF.Copy)
                        pC = ppc.tile([128, T], FP32, tag="pbc")
                        nc.tensor.matmul(pC[:], lhsT=t_selB64[64:80, :], rhs=dbc[64:80, :],
                                         start=True, stop=True)
                        Cbc = wkb.tile([128, T], FP32, tag="Cbc")
                        nc.scalar.activation(Cbc[:], pC[:], AF.Copy)

                        py = ppy.tile([ED, T], FP32, tag="psy")
                        for eo in range(4):
                            pdb = ppb.tile([128, T], FP32, tag="pdb")
                            nc.tensor.matmul(pdb[:], lhsT=t_selE[eo][:],
                                             rhs=delta[:], start=True, stop=True)
                            dA = wkb.tile([128, T], FP32, tag="dA")
                            nc.scalar.activation(dA[:], pdb[:], AF.Exp,
                                                 scale=t_acol[l][eo][:])
                            pub = ppb.tile([128, T], FP32, tag="pdb")
                            nc.tensor.matmul(pub[:], lhsT=t_selE[eo][:],
                                             rhs=u[:], start=True, stop=True)
                            BX = wkb.tile([128, T], FP32, tag="BX")
                            nc.vector.tensor_tensor(BX[:], pub[:], Bbc[:],
                                                    AL.mult)
                            h = wkb.tile([128, T], FP32, tag="h")
                            nc.vector.tensor_tensor_scan(
                                out=h[:], data0=dA[:], data1=BX[:],
                                initial=t_carry[b][eo][:],
                                op0=AL.mult, op1=AL.add)
                            nc.vector.tensor_copy(t_carry[b][eo][:],
                                                  h[:, T - 1:T])
                            Hc = wkb.tile([128, T], FP32, tag="Hc")
                            nc.vector.tensor_tensor(Hc[:], h[:], Cbc[:], AL.mult)
                            nc.tensor.matmul(py[:], lhsT=t_selS[eo][:],
                                             rhs=Hc[:], start=(eo == 0),
                                             stop=(eo == 3))
                        # y2 = xc*D + y ; gated = y2 * silu_z; out_proj + resid
                        y2 = wk.tile([2 * ED, T], FP32, tag="y2")
                        nc.vector.scalar_tensor_tensor(
                            out=y2[ED:, :], in0=sc[0:ED, :], scalar=t_dvec[l][:],
                            in1=py[:], op0=AL.mult, op1=AL.add)
                        gated = wk.tile([ED, T], FP32, tag="gated")
                        nc.vector.tensor_tensor(gated[:], y2[ED:, :], sc[ED:, :],
                                                AL.mult)
                        po = pps.tile([DM, T], FP32, tag="ps_small")
                        nc.tensor.matmul(po[:], lhsT=t_woutT[l][:], rhs=gated[:],
                                         start=True, stop=True)
                        nc.vector.tensor_tensor(r_out[b][:, s0:s1], po[:],
                                                r_in[b][:, s0:s1], AL.add)

            # ---------- epilogue: lin_out ----------
            for b in range(b_loc):
                for c in range(nchunk):
                    pf = pps.tile([1, T], FP32, tag="ps_small")
                    nc.tensor.matmul(pf[:], lhsT=t_wloT[:],
                                     rhs=t_res[b][:, c * T:(c + 1) * T],
                                     start=True, stop=True)
                    oc = wk.tile([1, T], FP32, tag="oc")
                    nc.scalar.activation(oc[:], pf[:], AF.Identity,
                                         bias=t_lob[:])
                    nc.sync.dma_start(out_d[b:b + 1, c * T:(c + 1) * T], oc[:])

    nc.finalize()
    return nc


def pack_weights(inputs):
    """Host-side packing of the tiny weight set into device layouts."""
    f = np.float32
    w = {}
    w["win_T"] = np.ascontiguousarray(inputs["lin_in_w"].T).astype(f)
    w["win_b"] = inputs["lin_in_b"].reshape(DM, 1).astype(f)
    w["wlo_T"] = np.ascontiguousarray(inputs["lin_out_w"].T).astype(f)
    w["lob"] = inputs["lin_out_b"].reshape(1, 1).astype(f)
    w["ident"] = np.eye(128, dtype=f)
    w["ones16"] = np.ones((DM, 1), f)
    w["ones_1x16"] = np.ones((1, DM), f)
    w["ones_1x32"] = np.ones((1, ED), f)
    selE = np.zeros((4, ED, 128), f)
    selS = np.zeros((4, 128, ED), f)
    for eo in range(4):
        for p in range(128):
            e = eo * 8 + p // 16
            selE[eo, e, p] = 1.0
            selS[eo, p, e] = 1.0
    w["selE"] = selE
    w["selS"] = selS
    selB = np.zeros((N, 128), f)
    for p in range(128):
        selB[p % 16, p] = 1.0
    w["selB"] = selB

    wcz = np.zeros((2, 4, DM, 2 * ED), f)
    sbias = np.zeros((2, 2 * ED, 1), f)
    wxT = np.zeros((2, ED, 80), f)
    dtw = np.zeros((2, ED, 1), f)
    dtb = np.zeros((2, ED, 1), f)
    acol = np.zeros((2, 4, 128, 1), f)
    dvec = np.zeros((2, ED, 1), f)
    woutT = np.zeros((2, ED, DM), f)
    for l in range(2):
        p = f"l{l}_"
        in_proj = inputs[p + "in_proj_w"].astype(f)      # [64, 16]
        norm_w = inputs[p + "norm_w"].astype(f)          # [16]
        ipw = in_proj * norm_w[None, :]                  # fold norm weight
        conv_w = inputs[p + "conv_w"].astype(f)          # [32, 4]
        for k in range(4):
            # xin part: rows of in_proj 0..31, scaled by conv_w[:, k]
            wcz[l, k, :, 0:ED] = (ipw[0:ED, :] * conv_w[:, k:k + 1]).T
            if k == 3:
                wcz[l, k, :, ED:] = ipw[ED:, :].T        # z passthrough
        sbias[l, 0:ED, 0] = inputs[p + "conv_b"].astype(f)
        xp = inputs[p + "x_proj_w"].astype(f)            # [33, 32]
        wxT[l][:, 0:1] = xp[0:1, :].T                    # r
        wxT[l][:, 32:48] = xp[1:17, :].T                 # B
        wxT[l][:, 64:80] = xp[17:33, :].T                # C
        dtw[l] = inputs[p + "dt_proj_w"].astype(f).reshape(ED, 1)
        dtb[l] = inputs[p + "dt_proj_b"].astype(f).reshape(ED, 1)
        A = -np.exp(inputs[p + "A_log"].astype(np.float64)).astype(f)  # [32,16]
        for eo in range(4):
            for q in range(128):
                acol[l, eo, q, 0] = A[eo * 8 + q // 16, q % 16]
        dvec[l] = inputs[p + "D"].astype(f).reshape(ED, 1)
        woutT[l] = inputs[p + "out_proj_w"].astype(f).T  # [32, 16]
    w["wcz"], w["sbias"], w["wx_T"] = wcz, sbias, wxT
    c2 = np.zeros((128, 2), f); c2[:, 0] = EPS; c2[:, 1] = 1.0 / DM
    w["cst"] = c2
    w["dtw"], w["dtb"], w["acol"], w["dvec"], w["wout_T"] = (
        dtw, dtb, acol, dvec, woutT)
    return w


_NC_CACHE = {}


def get_nc():
    if "nc" not in _NC_CACHE:
        _NC_CACHE["nc"] = build_nc()
    return _NC_CACHE["nc"]


def kernel(**inputs):
    nc = get_nc()
    w = pack_weights(inputs)
    x = np.asarray(inputs["x"], np.float32)          # [32, 4096, 64]
    n_cores = 8
    in_maps = []
    for c in range(n_cores):
        m = dict(w)
        m["x"] = np.ascontiguousarray(x[c * B_LOC:(c + 1) * B_LOC])
        in_maps.append(m)
    res = run_bass_kernel_spmd(nc, in_maps, list(range(n_cores)))
    outs = [res.results[c]["out"] for c in range(n_cores)]   # each [4, 4096]
    full = np.concatenate(outs, axis=0)                       # [32, 4096]
    return full.reshape(-1).astype(np.float32)
